# revision 11
# baseline (speedup 1.0000x reference)
"""Distributed 2-layer GCN (GCNConv -> ReLU -> GCNConv -> log_softmax) on 8
Trainium2 NeuronCores via Bass/Tile.

Sharding: nodes 1D-partitioned across the 8 cores (12500 each). Each core owns
the edges whose dst falls in its shard. Per core, edges are bucketed into 8
groups by src window (= owner core of src), ordered by local dst within fixed
512-dst chunks. Scaled features G = rsqrt(deg) * H are AllGathered; per-edge
messages are gathered from SBUF windows with gpsimd ap_gather, segment-summed
via a free-dim cumsum + end-position extraction + adjacent difference, and
group-partials are reduced on the TensorEngine.

v2 tuning (gpsimd ap_gather is ~28ns/idx and fully serializes the kernel):
 - per-chunk gather sizes (num_idxs = that chunk's max cell, not the global
   max) to cut padding idx,
 - the layer epilogue (+G, *invs, bias/relu) is fused per-chunk in [16, 512]
   staging space and the W2/log_softmax head is interleaved into the layer-2
   chunk loop, removing the serial tail after the last gather.
"""
import numpy as np

from concourse import bacc, mybir, tile
from concourse.bass_utils import run_bass_kernel_spmd
from concourse import masks

# ---------------- problem constants (hardcoded) ----------------
N = 100_000
NCORES = 8
SH = N // NCORES            # 12500 nodes per core
SH_PAD = 12544              # = 98*128, x rows padded
FOLD = 2048                 # folded layout: node n -> (16*(n//FOLD)+f, n%FOLD)
NBLK = 7                    # ceil(SH/FOLD)
PF = 16 * NBLK              # 112 partitions used by folded arrays
SPAN = 512                  # dsts per chunk
NCHUNK = 25                 # 24 full + 1 of 212
LAST_SPAN = SH - (NCHUNK - 1) * SPAN   # 212
LAST_NEND = 256             # last chunk end-slots padded to 256
NPAD = NBLK * FOLD          # 14336
WCOLS = 1 + NPAD            # gather window width (col 0 = zero pad)
HID = 16
NCLS = 64
FEAT = 512
F32 = mybir.dt.float32
F32R = mybir.dt.float32r
I16 = mybir.dt.int16
ADD = mybir.AluOpType.add
SUB = mybir.AluOpType.subtract
MULT = mybir.AluOpType.mult
AF = mybir.ActivationFunctionType


# ---------------- host-side prep (integer layout only) ----------------
def _wrap16(arr_per_group, cols):
    out = np.zeros((128, cols), dtype=np.int16)
    for g, a in enumerate(arr_per_group):
        j = np.arange(len(a))
        out[16 * g + (j % 16), j // 16] = a.astype(np.int16)
    return out


def prep(x, edge_index, W1, b1, W2, b2):
    src = np.asarray(edge_index[0], dtype=np.int64)
    dst = np.asarray(edge_index[1], dtype=np.int64)
    x = np.asarray(x, dtype=np.float32)
    W1 = np.asarray(W1, dtype=np.float32)
    b1 = np.asarray(b1, dtype=np.float32)
    W2 = np.asarray(W2, dtype=np.float32)
    b2 = np.asarray(b2, dtype=np.float32)

    owner = dst // SH
    g_all = src // SH
    sl_all = src % SH
    dl_all = dst % SH
    k_all = dl_all // SPAN

    order = np.lexsort((dl_all, k_all, g_all, owner))
    so, go, ko, dlo, slo = (
        owner[order], g_all[order], k_all[order], dl_all[order], sl_all[order])

    cell_id = (so * 8 + go) * NCHUNK + ko
    counts = np.bincount(cell_id, minlength=NCORES * 8 * NCHUNK).reshape(
        NCORES, 8, NCHUNK)
    # per-chunk caps: pad each chunk's gather only to that chunk's max cell
    caps = tuple(
        int(np.ceil((int(counts[:, :, k].max()) + 1) / 128) * 128)
        for k in range(NCHUNK))
    C = max(caps[:NCHUNK - 1])
    CL = caps[NCHUNK - 1]
    assert C < 32000 and CL < 32000

    cell_sizes = counts.reshape(-1)
    cell_starts = np.concatenate([[0], np.cumsum(cell_sizes)])[:-1]

    in_maps = []
    for c in range(NCORES):
        # feature-major layout (host-side permutation only) so phase 1 can
        # DMA [128-feat, 512-node] tiles directly instead of transposing
        # through the PE
        xc = np.zeros((FEAT, SH_PAD), dtype=np.float32)
        xc[:, :SH] = x[c * SH:(c + 1) * SH].T

        indeg = np.bincount(dl_all[owner == c], minlength=SH).astype(np.float32)
        cnt_pad = np.zeros(NPAD, dtype=np.float32)
        cnt_pad[:SH] = indeg
        cnt_folded = np.zeros((128, FOLD), dtype=np.float32)
        for b in range(NBLK):
            cnt_folded[16 * b:16 * b + 16] = cnt_pad[b * FOLD:(b + 1) * FOLD]

        eidx_main = np.zeros((NCHUNK - 1, 128, C // 16), dtype=np.int16)
        endp_main = np.zeros((NCHUNK - 1, 128, SPAN // 16), dtype=np.int16)
        eidx_last = np.zeros((128, CL // 16), dtype=np.int16)
        endp_last = np.zeros((128, LAST_NEND // 16), dtype=np.int16)

        for k in range(NCHUNK):
            last = k == NCHUNK - 1
            span = LAST_SPAN if last else SPAN
            n_end = LAST_NEND if last else SPAN
            idx_pg, end_pg = [], []
            for g in range(8):
                cid = (c * 8 + g) * NCHUNK + k
                s0, n = cell_starts[cid], cell_sizes[cid]
                sls = slo[s0:s0 + n]
                dls = dlo[s0:s0 + n] - k * SPAN
                a = np.zeros(1 + n, dtype=np.int64)
                a[1:] = 1 + sls
                idx_pg.append(a)
                ep = np.zeros(n_end, dtype=np.int64)
                if n > 0:
                    ep[:span] = np.searchsorted(dls, np.arange(span), side="right")
                if n_end > span:
                    ep[span:] = ep[span - 1] if span > 0 else 0
                end_pg.append(ep)
            cap = CL if last else C
            w_idx = _wrap16(idx_pg, cap // 16)
            w_end = _wrap16(end_pg, n_end // 16)
            if last:
                eidx_last, endp_last = w_idx, w_end
            else:
                eidx_main[k], endp_main[k] = w_idx, w_end

        sel = np.zeros((128, HID), dtype=np.float32)
        for g in range(8):
            sel[16 * g + np.arange(HID), np.arange(HID)] = 1.0
        b1s = b1.reshape(HID, 1).astype(np.float32)
        b2b8 = np.tile(b2.reshape(1, NCLS), (128, 4)).astype(np.float32)
        w2v = np.zeros((4, 128, NCLS), dtype=np.float32)
        for v in range(4):
            for m in range(2):
                w2v[v, 64 * m + 16 * v:64 * m + 16 * v + 16] = W2

        in_maps.append({
            "x": xc,
            "W1": W1,
            "W2": W2,
            "b1s": b1s,
            "w2v": w2v,
            "sel": sel,
            "b2b8": b2b8,
            "cntf": cnt_folded,
            "eidx_m": eidx_main,
            "endp_m": endp_main,
            "eidx_l": eidx_last,
            "endp_l": endp_last,
        })
    return in_maps, {"C": C, "CL": CL, "caps": caps}


# ---------------- device kernel ----------------
def build_nc(C, CL, caps):
    nc = bacc.Bacc("TRN2", target_bir_lowering=False, debug=False,
                   num_devices=NCORES)

    x_d = nc.dram_tensor("x", [FEAT, SH_PAD], F32R, kind="ExternalInput")
    w1_d = nc.dram_tensor("W1", [FEAT, HID], F32, kind="ExternalInput")
    w2_d = nc.dram_tensor("W2", [HID, NCLS], F32, kind="ExternalInput")
    b1_d = nc.dram_tensor("b1s", [HID, 1], F32, kind="ExternalInput")
    w2v_d = nc.dram_tensor("w2v", [4, 128, NCLS], F32, kind="ExternalInput")
    sel_d = nc.dram_tensor("sel", [128, HID], F32, kind="ExternalInput")
    b2_d = nc.dram_tensor("b2b8", [128, 4 * NCLS], F32, kind="ExternalInput")
    cnt_d = nc.dram_tensor("cntf", [128, FOLD], F32, kind="ExternalInput")
    eim_d = nc.dram_tensor("eidx_m", [NCHUNK - 1, 128, C // 16], I16,
                           kind="ExternalInput")
    epm_d = nc.dram_tensor("endp_m", [NCHUNK - 1, 128, SPAN // 16], I16,
                           kind="ExternalInput")
    eil_d = nc.dram_tensor("eidx_l", [128, CL // 16], I16, kind="ExternalInput")
    epl_d = nc.dram_tensor("endp_l", [128, LAST_NEND // 16], I16,
                           kind="ExternalInput")
    out_d = nc.dram_tensor("out", [SH_PAD, NCLS], F32, kind="ExternalOutput")

    rg = [list(range(NCORES))]

    with tile.TileContext(nc) as tc:
        with (
            tc.tile_pool(name="const", bufs=1) as cpool,
            tc.tile_pool(name="dram", bufs=1, space="DRAM") as dpool,
        ):
            # ---- constants ----
            ident = cpool.tile([128, 128], F32)
            masks.make_identity(nc, ident[:])
            sel = cpool.tile([128, HID], F32)
            nc.sync.dma_start(out=sel[:], in_=sel_d[:, :])
            w1sb = cpool.tile([128, 4, HID], F32)
            nc.sync.dma_start(out=w1sb[:], in_=w1_d[:, :].rearrange(
                "(j p) h -> p j h", p=128))
            w1r = cpool.tile([128, 4, HID], F32R)
            nc.vector.tensor_copy(w1r[:], w1sb[:])
            w2v_sb = cpool.tile([128, 4, NCLS], F32)
            nc.sync.dma_start(out=w2v_sb[:], in_=w2v_d[:, :, :].rearrange(
                "v p c -> p v c"))
            b1sb = cpool.tile([HID, 1], F32)
            nc.sync.dma_start(out=b1sb[:], in_=b1_d[:, :])
            b2sb = cpool.tile([128, 4 * NCLS], F32)
            nc.sync.dma_start(out=b2sb[:], in_=b2_d[:, :])

            invs = cpool.tile([128, FOLD], F32)
            tmpc = cpool.tile([128, FOLD], F32)
            nc.sync.dma_start(out=tmpc[:], in_=cnt_d[:, :])
            nc.scalar.activation(out=invs[:], in_=tmpc[:], func=AF.Sqrt,
                                 bias=1.0, scale=1.0)
            nc.vector.reciprocal(out=tmpc[:], in_=invs[:])
            invs = tmpc  # final rsqrt(deg)

            zerot = cpool.tile([128, max(C, CL)], F32)
            nc.vector.memset(zerot[:], 0.0)

            gf = cpool.tile([128, FOLD], F32)    # G (scaled features), folded
            af = cpool.tile([128, FOLD], F32)    # layer-2 A, folded
            nc.scalar.memzero(gf[:])
            nc.scalar.memzero(af[:])

            win = cpool.tile([128, WCOLS], F32)  # gather windows
            winb = cpool.tile([128, NPAD], mybir.dt.bfloat16)
            gfb = cpool.tile([128, FOLD], mybir.dt.bfloat16)

            contribs, galls = [], []
            for l in range(2):
                contrib_t = dpool.tile([PF, FOLD], mybir.dt.bfloat16,
                                       tag=f"contrib{l}")
                gall_t = dpool.tile([NCORES * PF, FOLD], mybir.dt.bfloat16,
                                    tag=f"gall{l}")
                contribs.append(contrib_t)
                galls.append(gall_t)

            # ---- phase 1: H1 = x @ W1 (transpose via PE), G1 = H1*invs ----
            with (
                tc.tile_pool(name="p1", bufs=4) as p1pool,
                tc.tile_pool(name="p1x", bufs=2) as p1xpool,
                tc.tile_pool(name="p1ph", bufs=2, space="PSUM") as p1ph,
            ):
                for st in range(25):
                    nn = 512 if st < 24 else 256
                    xt4 = p1xpool.tile([128, 4, 512], F32R, tag="xt4")
                    for j in range(4):
                        nc.sync.dma_start(
                            out=xt4[:, j, :nn],
                            in_=x_d[128 * j:128 * (j + 1),
                                    512 * st:512 * st + nn])
                    h1p = p1ph.tile([16, 512], F32, tag="h1p")
                    for j in range(4):
                        nc.tensor.matmul(
                            h1p[:, :nn],
                            lhsT=w1r[:, j, :],
                            rhs=xt4[:, j, :nn],
                            start=(j == 0), stop=(j == 3))
                    B, off = st // 4, 512 * (st % 4)
                    hstage = p1pool.tile([16, 512], F32, tag="hstage")
                    nc.any.tensor_copy(out=hstage[:, :nn], in_=h1p[:, :nn])
                    nc.sync.dma_start(
                        out=gf[16 * B:16 * B + 16, off:off + nn],
                        in_=hstage[:, :nn])

            # G1 = H1 * rsqrt(deg) (full-tile; block slices are not
            # legal compute operands)
            nc.vector.tensor_tensor(out=gf[:], in0=gf[:], in1=invs[:], op=MULT)

            # ---- two GCN layers (epilogue fused per chunk; head inline) ----
            with (
                tc.tile_pool(name="eg", bufs=2) as egpool,
                tc.tile_pool(name="es", bufs=2) as espool,
                tc.tile_pool(name="ei", bufs=2) as eipool,
                tc.tile_pool(name="ev", bufs=2) as evpool,
                tc.tile_pool(name="st", bufs=2) as stpool,
                tc.tile_pool(name="hd", bufs=1) as hdpool,
                tc.tile_pool(name="eps", bufs=3, space="PSUM") as epspool,
                tc.tile_pool(name="hps", bufs=2, space="PSUM") as hpsps,
            ):
                for layer in range(2):
                    # publish G, all-gather, load windows (split DMAs so all
                    # 16 queues carry the window load)
                    nc.vector.tensor_copy(out=gfb[:], in_=gf[:])
                    nc.sync.dma_start(out=contribs[layer][0:56, :],
                                      in_=gfb[0:56, :])
                    nc.sync.dma_start(out=contribs[layer][56:PF, :],
                                      in_=gfb[56:PF, :])
                    nc.gpsimd.collective_compute(
                        "AllGather", mybir.AluOpType.bypass,
                        replica_groups=rg,
                        ins=[contribs[layer][:].opt()],
                        outs=[galls[layer][:].opt()])
                    nc.vector.memset(win[:, 0:1], 0.0)
                    for g in range(8):
                        for b0, b1 in ((0, 3), (3, NBLK)):
                            nc.sync.dma_start(
                                out=winb[16 * g:16 * g + 16,
                                         b0 * FOLD:b1 * FOLD].rearrange(
                                    "f (b j) -> f b j", b=b1 - b0),
                                in_=galls[layer][PF * g + 16 * b0:
                                                 PF * g + 16 * b1,
                                                 :].rearrange(
                                    "(b f) j -> f b j", f=16))
                    nc.vector.tensor_copy(out=win[:, 1:WCOLS], in_=winb[:])

                    def emit_gather(k):
                        last = k == NCHUNK - 1
                        cap = caps[k]
                        et = eipool.tile([128, (CL if last else C) // 16], I16,
                                         tag="eil" if last else "eim")
                        nc.sync.dma_start(
                            out=et[:, :cap // 16],
                            in_=eil_d[:, :cap // 16] if last
                            else eim_d[k][:, :cap // 16])
                        gat = egpool.tile([128, CL if last else C], F32,
                                          tag="gatl" if last else "gat")
                        nc.gpsimd.ap_gather(
                            out_ap=gat[:, :cap], in_ap=win[:], idxs_ap=et[:, :cap // 16],
                            channels=128, num_elems=WCOLS, d=1, num_idxs=cap)
                        scan = espool.tile([128, CL if last else C], F32,
                                           tag="scanl" if last else "scan")
                        nc.vector.tensor_tensor_scan(
                            out=scan[:, :cap], data0=zerot[:, :cap],
                            data1=gat[:, :cap],
                            initial=0.0, op0=ADD, op1=ADD)
                        return k, cap, scan

                    # software pipeline: issue chunk k+1's main gather before
                    # chunk k's end-extraction so gpsimd never waits on the
                    # DVE scan
                    def emit_tail(st_):
                        k, cap, scan = st_
                        last = k == NCHUNK - 1
                        nend = LAST_NEND if last else SPAN
                        b = (k * SPAN) // FOLD
                        off = (k * SPAN) % FOLD
                        ept = eipool.tile([128, nend // 16], I16, tag="ept")
                        nc.sync.dma_start(
                            out=ept[:], in_=epl_d[:, :] if last else epm_d[k])
                        endv = evpool.tile([128, nend], F32, tag="endv")
                        nc.gpsimd.ap_gather(
                            out_ap=endv[:], in_ap=scan[:, :cap], idxs_ap=ept[:],
                            channels=128, num_elems=cap, d=1, num_idxs=nend)
                        redp = epspool.tile([16, nend], F32, tag="redp")
                        nc.tensor.matmul(redp[:, :], lhsT=sel[:, :],
                                         rhs=endv[:, :], start=True, stop=True)
                        # seg-sums via adjacent differences of the cumsum
                        rstage = evpool.tile([16, SPAN], F32, tag="rstage")
                        nc.any.tensor_copy(out=rstage[:, :nend],
                                           in_=redp[:, :nend])
                        dstage = evpool.tile([16, SPAN], F32, tag="dstage")
                        nc.any.tensor_copy(out=dstage[:, 0:1],
                                           in_=rstage[:, 0:1])
                        nc.any.tensor_tensor(
                            out=dstage[:, 1:nend],
                            in0=rstage[:, 1:nend], in1=rstage[:, 0:nend - 1],
                            op=SUB)
                        # fused per-chunk epilogue: A = invs*(seg + G) [+b, relu]
                        gfst = stpool.tile([16, SPAN], F32, tag="gfst")
                        nc.sync.dma_start(out=gfst[:, :nend],
                                          in_=gf[16 * b:16 * b + 16,
                                               off:off + nend])
                        ivst = stpool.tile([16, SPAN], F32, tag="ivst")
                        nc.sync.dma_start(out=ivst[:, :nend],
                                          in_=invs[16 * b:16 * b + 16,
                                               off:off + nend])
                        nc.vector.tensor_tensor(out=dstage[:, :nend],
                                                in0=dstage[:, :nend],
                                                in1=gfst[:, :nend], op=ADD)
                        nc.vector.tensor_tensor(out=dstage[:, :nend],
                                                in0=dstage[:, :nend],
                                                in1=ivst[:, :nend], op=MULT)
                        if layer == 0:
                            # A1 = relu(. + b1); G2 = A1 * invs -> gf
                            nc.scalar.activation(out=dstage[:, :nend],
                                                 in_=dstage[:, :nend],
                                                 func=AF.Relu,
                                                 bias=b1sb[:, 0:1], scale=1.0)
                            nc.vector.tensor_tensor(out=dstage[:, :nend],
                                                    in0=dstage[:, :nend],
                                                    in1=ivst[:, :nend],
                                                    op=MULT)
                            nc.sync.dma_start(
                                out=gf[16 * b:16 * b + 16, off:off + nend],
                                in_=dstage[:, :nend])
                        else:
                            # out2 = A2; stage to af, then head for 4 blocks
                            nc.sync.dma_start(
                                out=af[16 * b:16 * b + 16, off:off + nend],
                                in_=dstage[:, :nend])
                            nblk = 4 if not last else 2
                            osb = hdpool.tile([128, 4 * NCLS], F32, tag="osb")
                            esb = hdpool.tile([128, 4 * NCLS], F32, tag="esb")
                            lns = hdpool.tile([128, 4], F32, tag="lns")
                            for cc in range(nblk):
                                qq = 4 * k + cc
                                B2, ch = qq // 16, qq % 16
                                o2p = hpsps.tile([128, NCLS], F32, tag="o2p")
                                base, v = (0 if B2 < 4 else 64), B2 % 4
                                nc.tensor.matmul(
                                    o2p[:, :],
                                    lhsT=af[base:base + 64,
                                            128 * ch:128 * (ch + 1)],
                                    rhs=w2v_sb[base:base + 64, v, :],
                                    start=True, stop=True)
                                nc.any.tensor_tensor(
                                    out=osb[:, NCLS * cc:NCLS * (cc + 1)],
                                    in0=o2p[:, :], in1=b2sb[:, 0:NCLS], op=ADD)
                            nb = nblk * NCLS
                            nc.scalar.activation(out=esb[:, :nb],
                                                 in_=osb[:, :nb], func=AF.Exp)
                            nc.vector.tensor_reduce(
                                out=lns[:, :nblk],
                                in_=esb[:, :nb].rearrange(
                                    "p (j c) -> p j c", c=NCLS),
                                axis=mybir.AxisListType.X, op=ADD)
                            nc.scalar.activation(out=lns[:, :nblk],
                                                 in_=lns[:, :nblk], func=AF.Ln)
                            for cc in range(nblk):
                                nc.vector.tensor_scalar(
                                    out=osb[:, NCLS * cc:NCLS * (cc + 1)],
                                    in0=osb[:, NCLS * cc:NCLS * (cc + 1)],
                                    scalar1=lns[:, cc:cc + 1], scalar2=None,
                                    op0=SUB)
                            nc.sync.dma_start(
                                out=out_d[512 * k:512 * k + 128 * nblk,
                                          :].rearrange(
                                    "(j p) c -> p j c", p=128),
                                in_=osb[:, :nb].rearrange(
                                    "p (j c) -> p j c", c=NCLS))

                    pend = emit_gather(0)
                    for k in range(1, NCHUNK):
                        nxt = emit_gather(k)
                        emit_tail(pend)
                        pend = nxt
                    emit_tail(pend)

    nc.compile()
    return nc


_CACHE = {}


def kernel(x, edge_index, W1, b1, W2, b2):
    in_maps, meta = prep(x, edge_index, W1, b1, W2, b2)
    key = (meta["C"], meta["CL"], meta["caps"])
    if key not in _CACHE:
        _CACHE[key] = build_nc(*key)
    nc = _CACHE[key]
    res = run_bass_kernel_spmd(nc, in_maps, list(range(NCORES)))
    out = np.concatenate([res.results[c]["out"][:SH] for c in range(NCORES)],
                         axis=0).astype(np.float32)
    kernel._last_exec_time_ns = res.exec_time_ns
    return out
